# revision 1
# baseline (speedup 1.0000x reference)
"""DeepSeekMoE (B=2,S=2048,H=1024,I=2816, 7 routed experts top-2 + 1 shared) on 8 trn2 NeuronCores.

Strategy: expert-parallel sparse dispatch.
  - Host computes the router (fp32, 0.01% of FLOPs) and dispatches tokens:
    core c (c<7) owns routed expert c; the largest expert's token list is split
    with core 7 to balance load. Every core also computes the shared expert for
    its 512-token output slice.
  - Each core receives: gathered+transposed bf16 activations for its token list,
    its expert's weights (bf16), scatter indices and combine weights.
  - Device: SwiGLU MLP for the routed tokens -> scale by combine weight ->
    indirect-DMA scatter into a [4096,1024] fp32 partial (pads dropped via
    bounds_check), ReduceScatter(add) over the 8 cores (overlapped with the
    shared-expert MLP), then out = rs_out + shared_y.
  - Host concatenates the 8 [512,1024] output shards.
"""

import math
import os
import sys
import types

import numpy as np
import ml_dtypes

for _p in ('/opt/trn_rl_repo', '/root/.axon_site/_ro/trn_rl_repo'):
    if os.path.isdir(_p) and _p not in sys.path:
        sys.path.append(_p)


def _install_profile_glue():
    """Optional: register the NTFF profile hook so trace=True/BASS_TRACE works
    under axon (the image's antenv lacks axon_hooks). Harmless if unavailable."""
    try:
        import antenv
        if 'antenv.axon_hooks' in sys.modules:
            return
        mod = types.ModuleType('antenv.axon_hooks')
        holder = [None]
        mod.set_axon_ntff_profile_hook = lambda h: holder.__setitem__(0, h)
        mod.get_axon_ntff_profile_hook = lambda: holder[0]
        sys.modules['antenv.axon_hooks'] = mod
        antenv.axon_hooks = mod
        so = '/opt/axon/libaxon_pjrt.so'
        if os.path.exists(so):
            from trn_agent_boot.trn_boot import _ntff_profile_via_ctypes
            hook = _ntff_profile_via_ctypes(so)
            if hook is not None:
                mod.set_axon_ntff_profile_hook(hook)
    except Exception:
        pass


_install_profile_glue()

import concourse.bass as bass
import concourse.mybir as mybir
from concourse.bass_utils import run_bass_kernel_spmd
from concourse.tile import TileContext

B, S, H, I = 2, 2048, 1024, 2816
E_ROUTED = 7
TOP_K = 2
T = B * S                  # 4096 tokens
NCORES = 8
SH = T // NCORES           # 512 shared-slice tokens per core
KH = H // 128              # 8 contraction chunks over H
KI = I // 128              # 22 contraction chunks over I
NH = H // 512              # 2 N-chunks for the down matmul

F32 = mybir.dt.float32
BF16 = mybir.dt.bfloat16
I32 = mybir.dt.int32

PAD_IDX = 1 << 20          # scatter index for pad slots; dropped by bounds_check

LAST_RESULT = None         # BassKernelResults of the most recent run (for tests)

_PROG_CACHE = {}


def _split_sync_waits(nc, max_waits=1):
    """This container's walrus rejects >1 sync wait per instruction; spill
    extra waits onto same-engine NoOps placed just before the instruction."""
    for f in nc.m.functions:
        for bb in f.blocks:
            new_list = []
            changed = False
            for inst in bb.instructions:
                si = inst.sync_info
                if si is not None and si.on_wait is not None and len(si.on_wait) > max_waits:
                    waits = list(si.on_wait)
                    while len(waits) > max_waits:
                        chunk, waits = waits[:max_waits], waits[max_waits:]
                        nop = mybir.InstNoOp(
                            name=nc.get_next_instruction_name(),
                            engine=inst.engine, bass_nofuse=True,
                            sync_info=mybir.SyncInfo(on_wait=chunk, on_update=[]),
                        )
                        new_list.append(nop)
                    inst.sync_info = mybir.SyncInfo(
                        on_wait=waits, on_update=list(si.on_update or []))
                    changed = True
                new_list.append(inst)
            if changed:
                bb.instructions[:] = new_list


def _col_tiles(total, width=512):
    """[(start, size), ...] covering `total` columns in <=width chunks."""
    out = []
    c = 0
    while c < total:
        out.append((c, min(width, total - c)))
        c += width
    return out


def _build_program(C):
    """Build the SPMD bass program for routed capacity C (multiple of 128)."""
    CT = C + SH
    NM = C // 128          # routed 128-token row chunks

    nc = bass.Bass()
    xt = nc.declare_dram_parameter('xt', [H, CT], BF16, isOutput=False)
    idx = nc.declare_dram_parameter('idx', [C], I32, isOutput=False)
    wv = nc.declare_dram_parameter('wv', [C], F32, isOutput=False)
    gw = nc.declare_dram_parameter('gw', [H, I], BF16, isOutput=False)
    uw = nc.declare_dram_parameter('uw', [H, I], BF16, isOutput=False)
    dw = nc.declare_dram_parameter('dw', [I, H], BF16, isOutput=False)
    sgw = nc.declare_dram_parameter('sgw', [H, I], BF16, isOutput=False)
    suw = nc.declare_dram_parameter('suw', [H, I], BF16, isOutput=False)
    sdw = nc.declare_dram_parameter('sdw', [I, H], BF16, isOutput=False)
    out = nc.declare_dram_parameter('out', [SH, H], F32, isOutput=True)

    partial = nc.dram_tensor('partial', [T, H], F32)
    rs_out = nc.dram_tensor('rs_out', [SH, H], F32)

    with TileContext(nc) as tc:
        with (
            tc.tile_pool(name='big', bufs=1) as bigp,
            tc.tile_pool(name='wstream', bufs=2) as wsp,
            tc.tile_pool(name='work', bufs=2) as wkp,
            tc.tile_pool(name='psgu', bufs=2, space='PSUM') as psgu,
            tc.tile_pool(name='psy', bufs=2, space='PSUM') as psy,
        ):
            XT = bigp.tile([128, KH, CT], BF16, tag='XT')
            nc.sync.dma_start(out=XT[:, :, :], in_=xt.rearrange('(k p) c -> p k c', p=128))
            HT = bigp.tile([128, KI, CT], BF16, tag='HT')
            DW = bigp.tile([128, KI, H], BF16, tag='DW')
            IT = bigp.tile([128, NM], I32, tag='IT')
            nc.sync.dma_start(out=IT[:, :], in_=idx.rearrange('(m p) -> p m', p=128))
            WT = bigp.tile([128, NM], F32, tag='WT')
            nc.sync.dma_start(out=WT[:, :], in_=wv.rearrange('(m p) -> p m', p=128))
            YS = bigp.tile([128, SH // 128, H], F32, tag='YS')

            # zero the scatter target
            ZT = bigp.tile([128, H], F32, tag='ZT')
            nc.vector.memset(ZT[:, :], 0.0)
            for r in range(T // 128):
                nc.sync.dma_start(out=partial[r * 128:(r + 1) * 128, :], in_=ZT[:, :])

            def mlp_gate_up(gsrc, usrc, col0, cols):
                """h[:, col0:col0+cols] = silu(x@g) * (x@u), bf16, [I, tokens] layout."""
                g_r = gsrc.rearrange('(k p) i -> p k i', p=128)
                u_r = usrc.rearrange('(k p) i -> p k i', p=128)
                for i in range(KI):
                    gch = wsp.tile([128, KH, 128], BF16, tag='gch')
                    nc.sync.dma_start(out=gch[:, :, :], in_=g_r[:, :, i * 128:(i + 1) * 128])
                    uch = wsp.tile([128, KH, 128], BF16, tag='uch')
                    nc.sync.dma_start(out=uch[:, :, :], in_=u_r[:, :, i * 128:(i + 1) * 128])
                    for (t0, tn) in _col_tiles(cols):
                        gps = psgu.tile([128, 512], F32, tag='gps')
                        ups = psgu.tile([128, 512], F32, tag='ups')
                        for k in range(KH):
                            nc.tensor.matmul(
                                gps[:, :tn], lhsT=gch[:, k, :],
                                rhs=XT[:, k, col0 + t0:col0 + t0 + tn],
                                start=(k == 0), stop=(k == KH - 1))
                        for k in range(KH):
                            nc.tensor.matmul(
                                ups[:, :tn], lhsT=uch[:, k, :],
                                rhs=XT[:, k, col0 + t0:col0 + t0 + tn],
                                start=(k == 0), stop=(k == KH - 1))
                        at = wkp.tile([128, 512], F32, tag='act')
                        nc.scalar.activation(
                            out=at[:, :tn], in_=gps[:, :tn],
                            func=mybir.ActivationFunctionType.Silu)
                        nc.vector.tensor_tensor(
                            out=HT[:, i, col0 + t0:col0 + t0 + tn],
                            in0=at[:, :tn], in1=ups[:, :tn],
                            op=mybir.AluOpType.mult)

            def mlp_down(dsrc, col0, nchunks, sink):
                """y rows (128-token chunks m) = h[:, col0+...].T @ d; sink(m, psum)."""
                for k in range(KI):
                    nc.sync.dma_start(out=DW[:, k, :], in_=dsrc[k * 128:(k + 1) * 128, :])
                for m in range(nchunks):
                    yps = psy.tile([128, H], F32, tag='yps')
                    for n in range(NH):
                        for k in range(KI):
                            nc.tensor.matmul(
                                yps[:, n * 512:(n + 1) * 512],
                                lhsT=HT[:, k, col0 + m * 128:col0 + (m + 1) * 128],
                                rhs=DW[:, k, n * 512:(n + 1) * 512],
                                start=(k == 0), stop=(k == KI - 1))
                    sink(m, yps)

            # --- routed expert ---
            mlp_gate_up(gw, uw, 0, C)

            def routed_sink(m, yps):
                ysb = wkp.tile([128, H], F32, tag='ysb')
                nc.vector.tensor_scalar_mul(ysb[:, :], yps[:, :], WT[:, m:m + 1])
                nc.gpsimd.indirect_dma_start(
                    out=partial[:, :],
                    out_offset=bass.IndirectOffsetOnAxis(ap=IT[:, m:m + 1], axis=0),
                    in_=ysb[:, :], in_offset=None,
                    bounds_check=T - 1, oob_is_err=False)

            mlp_down(dw, 0, NM, routed_sink)

            # --- combine across cores (overlaps with the shared expert below) ---
            nc.gpsimd.collective_compute(
                'ReduceScatter', mybir.AluOpType.add,
                replica_groups=[list(range(NCORES))],
                ins=[partial[:, :]], outs=[rs_out[:, :]])

            # --- shared expert on this core's output slice ---
            mlp_gate_up(sgw, suw, C, SH)

            def shared_sink(m, yps):
                nc.vector.tensor_copy(out=YS[:, m, :], in_=yps[:, :])

            mlp_down(sdw, C, SH // 128, shared_sink)

            # --- out = rs_out + shared_y ---
            for m in range(SH // 128):
                rt = wkp.tile([128, H], F32, tag='rt')
                nc.sync.dma_start(out=rt[:, :], in_=rs_out[m * 128:(m + 1) * 128, :])
                ft = wkp.tile([128, H], F32, tag='ft')
                nc.vector.tensor_add(out=ft[:, :], in0=rt[:, :], in1=YS[:, m, :])
                nc.sync.dma_start(out=out[m * 128:(m + 1) * 128, :], in_=ft[:, :])

    _split_sync_waits(nc)
    return nc


def _dispatch(x2, router_w, routing_bias):
    """Host router + dispatch. Returns per-core (expert_id, token_idx, weight)."""
    logits = x2 @ router_w + routing_bias            # [T, 7] fp32
    # top-2 by logits (sigmoid is monotone); stable sort ties like jax top_k
    order = np.argsort(-logits, axis=1, kind='stable')[:, :TOP_K]
    probs = 1.0 / (1.0 + np.exp(-logits))
    rows = np.arange(T)
    s = probs[rows[:, None], order]                  # [T, 2]
    w = s / s.sum(axis=1, keepdims=True)             # renormalized combine weights

    lists = [[] for _ in range(E_ROUTED)]
    for k in range(TOP_K):
        for t, e, wt in zip(rows, order[:, k], w[:, k]):
            lists[e].append((t, wt))

    loads = np.array([len(l) for l in lists])
    emax = int(np.argmax(loads))
    half = len(lists[emax]) // 2
    core_specs = []
    for c in range(E_ROUTED):
        if c == emax:
            core_specs.append((c, lists[c][:half]))
        else:
            core_specs.append((c, lists[c]))
    core_specs.append((emax, lists[emax][half:]))
    return core_specs


def kernel(x, router_w, routing_bias, shared_gate, shared_up, shared_down,
           routed_gate, routed_up, routed_down):
    global LAST_RESULT
    x = np.asarray(x, np.float32)
    router_w = np.asarray(router_w, np.float32)
    routing_bias = np.asarray(routing_bias, np.float32)
    x2 = x.reshape(T, H)

    core_specs = _dispatch(x2, router_w, routing_bias)
    C = max(128, ((max(len(s[1]) for s in core_specs) + 127) // 128) * 128)

    bf = ml_dtypes.bfloat16
    routed_gate_b = np.asarray(routed_gate).astype(bf)
    routed_up_b = np.asarray(routed_up).astype(bf)
    routed_down_b = np.asarray(routed_down).astype(bf)
    sgw_b = np.ascontiguousarray(np.asarray(shared_gate).astype(bf))
    suw_b = np.ascontiguousarray(np.asarray(shared_up).astype(bf))
    sdw_b = np.ascontiguousarray(np.asarray(shared_down).astype(bf))

    in_maps = []
    for c in range(NCORES):
        e, toks = core_specs[c]
        n = len(toks)
        idx_h = np.full((C,), PAD_IDX, np.int32)
        wv_h = np.zeros((C,), np.float32)
        xg = np.zeros((C, H), np.float32)
        if n:
            tok_ids = np.fromiter((t for t, _ in toks), np.int64, n)
            idx_h[:n] = tok_ids
            wv_h[:n] = np.fromiter((wt for _, wt in toks), np.float64, n)
            xg[:n] = x2[tok_ids]
        xt_all = np.concatenate(
            [xg.T, x2[c * SH:(c + 1) * SH].T], axis=1).astype(bf)
        in_maps.append({
            'xt': np.ascontiguousarray(xt_all),
            'idx': idx_h,
            'wv': wv_h,
            'gw': np.ascontiguousarray(routed_gate_b[e]),
            'uw': np.ascontiguousarray(routed_up_b[e]),
            'dw': np.ascontiguousarray(routed_down_b[e]),
            'sgw': sgw_b, 'suw': suw_b, 'sdw': sdw_b,
        })

    nc = _PROG_CACHE.get(C)
    if nc is None:
        nc = _build_program(C)
        _PROG_CACHE[C] = nc

    res = run_bass_kernel_spmd(nc, in_maps, list(range(NCORES)))
    LAST_RESULT = res

    out = np.concatenate([res.results[c]['out'] for c in range(NCORES)], axis=0)
    return out.reshape(B, S, H).astype(np.float32)


# revision 4
# speedup vs baseline: 1.1068x; 1.1068x over previous
"""DeepSeekMoE (B=2,S=2048,H=1024,I=2816, 7 routed experts top-2 + 1 shared) on 8 trn2 NeuronCores.

Strategy: expert-parallel sparse dispatch.
  - Host computes the router (fp32, 0.01% of FLOPs) and dispatches tokens:
    core c (c<7) owns routed expert c; the largest expert's token list is split
    with core 7 to balance load. Every core also computes the shared expert for
    its 512-token output slice.
  - Each core receives: gathered+transposed bf16 activations for its token list,
    its expert's weights (bf16, chunk-shuffled for contiguous DMA), scatter
    indices and combine weights.
  - Device: SwiGLU MLP for the routed tokens -> scale by combine weight ->
    indirect-DMA scatter into a [4096,1024] bf16 partial (pads dropped via
    bounds_check), ReduceScatter(add) over the 8 cores (overlapped with the
    shared-expert MLP), then out = rs_out + shared_y.
  - Host concatenates the 8 [512,1024] output shards.
"""

import math
import os
import sys
import types

import numpy as np
import ml_dtypes

for _p in ('/opt/trn_rl_repo', '/root/.axon_site/_ro/trn_rl_repo'):
    if os.path.isdir(_p) and _p not in sys.path:
        sys.path.append(_p)


def _install_profile_glue():
    """Optional: register the NTFF profile hook so trace=True/BASS_TRACE works
    under axon (the image's antenv lacks axon_hooks). Harmless if unavailable."""
    try:
        import antenv
        if 'antenv.axon_hooks' in sys.modules:
            return
        mod = types.ModuleType('antenv.axon_hooks')
        holder = [None]
        mod.set_axon_ntff_profile_hook = lambda h: holder.__setitem__(0, h)
        mod.get_axon_ntff_profile_hook = lambda: holder[0]
        sys.modules['antenv.axon_hooks'] = mod
        antenv.axon_hooks = mod
        so = '/opt/axon/libaxon_pjrt.so'
        if os.path.exists(so):
            from trn_agent_boot.trn_boot import _ntff_profile_via_ctypes
            hook = _ntff_profile_via_ctypes(so)
            if hook is not None:
                mod.set_axon_ntff_profile_hook(hook)
    except Exception:
        pass


_install_profile_glue()

import concourse.bass as bass
import concourse.mybir as mybir
from concourse.bass_utils import run_bass_kernel_spmd
from concourse.tile import TileContext

B, S, H, I = 2, 2048, 1024, 2816
E_ROUTED = 7
TOP_K = 2
T = B * S                  # 4096 tokens
NCORES = 8
SH = T // NCORES           # 512 shared-slice tokens per core
KH = H // 128              # 8 contraction chunks over H
KI = I // 128              # 22 contraction chunks over I
NH = H // 512              # 2 N-chunks for the down matmul

F32 = mybir.dt.float32
BF16 = mybir.dt.bfloat16
I32 = mybir.dt.int32

PAD_IDX = 1 << 20          # scatter index for pad slots; dropped by bounds_check

LAST_RESULT = None         # BassKernelResults of the most recent run (for tests)

_PROG_CACHE = {}


def _split_sync_waits(nc, max_waits=1):
    """This container's walrus rejects >1 sync wait per instruction; spill
    extra waits onto same-engine NoOps placed just before the instruction."""
    for f in nc.m.functions:
        for bb in f.blocks:
            new_list = []
            changed = False
            for inst in bb.instructions:
                si = inst.sync_info
                if si is not None and si.on_wait is not None and len(si.on_wait) > max_waits:
                    waits = list(si.on_wait)
                    while len(waits) > max_waits:
                        chunk, waits = waits[:max_waits], waits[max_waits:]
                        nop = mybir.InstNoOp(
                            name=nc.get_next_instruction_name(),
                            engine=inst.engine, bass_nofuse=True,
                            sync_info=mybir.SyncInfo(on_wait=chunk, on_update=[]),
                        )
                        new_list.append(nop)
                    inst.sync_info = mybir.SyncInfo(
                        on_wait=waits, on_update=list(si.on_update or []))
                    changed = True
                new_list.append(inst)
            if changed:
                bb.instructions[:] = new_list


def _col_tiles(total, width=512):
    out = []
    c = 0
    while c < total:
        out.append((c, min(width, total - c)))
        c += width
    return out


def _build_program(C):
    """Build the SPMD bass program for routed capacity C (multiple of 128)."""
    CT = C + SH
    NM = C // 128          # routed 128-token row chunks

    nc = bass.Bass()
    xt = nc.declare_dram_parameter('xt', [H, CT], BF16, isOutput=False)
    idx = nc.declare_dram_parameter('idx', [C], I32, isOutput=False)
    wv = nc.declare_dram_parameter('wv', [C], F32, isOutput=False)
    # gate/up weights arrive chunk-shuffled: [KI, 128, KH, 128] so each
    # per-I-chunk stream DMA reads 2KB-contiguous per partition.
    gw = nc.declare_dram_parameter('gw', [KI, 128, KH, 128], BF16, isOutput=False)
    uw = nc.declare_dram_parameter('uw', [KI, 128, KH, 128], BF16, isOutput=False)
    dw = nc.declare_dram_parameter('dw', [I, H], BF16, isOutput=False)
    sgw = nc.declare_dram_parameter('sgw', [KI, 128, KH, 128], BF16, isOutput=False)
    suw = nc.declare_dram_parameter('suw', [KI, 128, KH, 128], BF16, isOutput=False)
    sdw = nc.declare_dram_parameter('sdw', [I, H], BF16, isOutput=False)
    out = nc.declare_dram_parameter('out', [SH, H], F32, isOutput=True)

    partial = nc.dram_tensor('partial', [T, H], BF16)
    rs_out = nc.dram_tensor('rs_out', [SH, H], BF16)

    with TileContext(nc) as tc:
        with (
            tc.tile_pool(name='big', bufs=1) as bigp,
            tc.tile_pool(name='wstream', bufs=3) as wsp,
            tc.tile_pool(name='work', bufs=2) as wkp,
            tc.tile_pool(name='psgu', bufs=2, space='PSUM') as psgu,
            tc.tile_pool(name='psy', bufs=2, space='PSUM') as psy,
        ):
            XT = bigp.tile([128, KH, CT], BF16, tag='XT')
            nc.sync.dma_start(out=XT[:, :, :], in_=xt.rearrange('(k p) c -> p k c', p=128))
            HT = bigp.tile([128, KI, CT], BF16, tag='HT')
            DW = bigp.tile([128, KI, H], BF16, tag='DW')
            IT = bigp.tile([128, NM], I32, tag='IT')
            nc.sync.dma_start(out=IT[:, :], in_=idx.rearrange('(m p) -> p m', p=128))
            WT = bigp.tile([128, NM], F32, tag='WT')
            nc.sync.dma_start(out=WT[:, :], in_=wv.rearrange('(m p) -> p m', p=128))
            YS = bigp.tile([128, SH // 128, H], F32, tag='YS')
            ZT = bigp.tile([128, H], BF16, tag='ZT')

            def mlp_gate_up(gsrc, usrc, col0, cols, after_first_i=None):
                """h[:, col0:col0+cols] = silu(x@g) * (x@u), bf16, [I, tokens] layout."""
                for i in range(KI):
                    gch = wsp.tile([128, KH, 128], BF16, tag='gch')
                    nc.sync.dma_start(out=gch[:, :, :], in_=gsrc[i, :, :, :])
                    uch = wsp.tile([128, KH, 128], BF16, tag='uch')
                    nc.sync.dma_start(out=uch[:, :, :], in_=usrc[i, :, :, :])
                    for (t0, tn) in _col_tiles(cols):
                        gps = psgu.tile([128, 512], F32, tag='gps')
                        ups = psgu.tile([128, 512], F32, tag='ups')
                        for k in range(KH):
                            nc.tensor.matmul(
                                gps[:, :tn], lhsT=gch[:, k, :],
                                rhs=XT[:, k, col0 + t0:col0 + t0 + tn],
                                start=(k == 0), stop=(k == KH - 1))
                        for k in range(KH):
                            nc.tensor.matmul(
                                ups[:, :tn], lhsT=uch[:, k, :],
                                rhs=XT[:, k, col0 + t0:col0 + t0 + tn],
                                start=(k == 0), stop=(k == KH - 1))
                        at = wkp.tile([128, 512], F32, tag='act')
                        nc.scalar.activation(
                            out=at[:, :tn], in_=gps[:, :tn],
                            func=mybir.ActivationFunctionType.Silu)
                        nc.vector.tensor_tensor(
                            out=HT[:, i, col0 + t0:col0 + t0 + tn],
                            in0=at[:, :tn], in1=ups[:, :tn],
                            op=mybir.AluOpType.mult)
                    if i == 0 and after_first_i is not None:
                        after_first_i()

            def mlp_down(dsrc, col0, nchunks, sink):
                for k in range(KI):
                    nc.sync.dma_start(out=DW[:, k, :], in_=dsrc[k * 128:(k + 1) * 128, :])
                for m in range(nchunks):
                    yps = psy.tile([128, H], F32, tag='yps')
                    for n in range(NH):
                        for k in range(KI):
                            nc.tensor.matmul(
                                yps[:, n * 512:(n + 1) * 512],
                                lhsT=HT[:, k, col0 + m * 128:col0 + (m + 1) * 128],
                                rhs=DW[:, k, n * 512:(n + 1) * 512],
                                start=(k == 0), stop=(k == KI - 1))
                    sink(m, yps)

            # --- routed expert ---
            def emit_zero_init():
                # deferred so startup DMA goes to weights first; gpsimd queue
                # to stay off the weight-stream (sync) queue
                nc.vector.memset(ZT[:, :], 0.0)
                for r in range(T // 128):
                    nc.gpsimd.dma_start(out=partial[r * 128:(r + 1) * 128, :], in_=ZT[:, :])

            mlp_gate_up(gw, uw, 0, C, after_first_i=emit_zero_init)

            def routed_sink(m, yps):
                ysb = wkp.tile([128, H], BF16, tag='ysb')
                nc.vector.tensor_scalar_mul(ysb[:, :], yps[:, :], WT[:, m:m + 1])
                nc.gpsimd.indirect_dma_start(
                    out=partial[:, :],
                    out_offset=bass.IndirectOffsetOnAxis(ap=IT[:, m:m + 1], axis=0),
                    in_=ysb[:, :], in_offset=None,
                    bounds_check=T - 1, oob_is_err=False)

            mlp_down(dw, 0, NM, routed_sink)

            # --- combine across cores (overlaps with the shared expert below) ---
            nc.gpsimd.collective_compute(
                'ReduceScatter', mybir.AluOpType.add,
                replica_groups=[list(range(NCORES))],
                ins=[partial[:, :]], outs=[rs_out[:, :]])

            # --- shared expert on this core's output slice ---
            mlp_gate_up(sgw, suw, C, SH)

            def shared_sink(m, yps):
                nc.vector.tensor_copy(out=YS[:, m, :], in_=yps[:, :])

            mlp_down(sdw, C, SH // 128, shared_sink)

            # --- out = rs_out + shared_y ---
            for m in range(SH // 128):
                rt = wkp.tile([128, H], BF16, tag='rt')
                nc.sync.dma_start(out=rt[:, :], in_=rs_out[m * 128:(m + 1) * 128, :])
                rtf = wkp.tile([128, H], F32, tag='rtf')
                nc.vector.tensor_copy(out=rtf[:, :], in_=rt[:, :])
                ft = wkp.tile([128, H], F32, tag='ft')
                nc.vector.tensor_add(out=ft[:, :], in0=rtf[:, :], in1=YS[:, m, :])
                nc.sync.dma_start(out=out[m * 128:(m + 1) * 128, :], in_=ft[:, :])

    _split_sync_waits(nc)
    return nc


def _dispatch(x2, router_w, routing_bias):
    """Host router + dispatch. Returns per-core (expert_id, token_idx, weight)."""
    logits = x2 @ router_w + routing_bias            # [T, 7] fp32
    order = np.argsort(-logits, axis=1, kind='stable')[:, :TOP_K]
    probs = 1.0 / (1.0 + np.exp(-logits))
    rows = np.arange(T)
    s = probs[rows[:, None], order]                  # [T, 2]
    w = s / s.sum(axis=1, keepdims=True)             # renormalized combine weights

    lists = [[] for _ in range(E_ROUTED)]
    for k in range(TOP_K):
        for t, e, wt in zip(rows, order[:, k], w[:, k]):
            lists[e].append((t, wt))

    loads = np.array([len(l) for l in lists])
    emax = int(np.argmax(loads))
    half = len(lists[emax]) // 2
    core_specs = []
    for c in range(E_ROUTED):
        if c == emax:
            core_specs.append((c, lists[c][:half]))
        else:
            core_specs.append((c, lists[c]))
    core_specs.append((emax, lists[emax][half:]))
    return core_specs


def _shuffle_gateup(wmat):
    """[H, I] -> [KI, 128(H-part), KH, 128(I-cols)] bf16, so the per-I-chunk
    stream DMA reads 2KB contiguous per partition."""
    return np.ascontiguousarray(
        wmat.reshape(KH, 128, KI, 128).transpose(2, 1, 0, 3).astype(ml_dtypes.bfloat16))


def kernel(x, router_w, routing_bias, shared_gate, shared_up, shared_down,
           routed_gate, routed_up, routed_down):
    global LAST_RESULT
    x = np.asarray(x, np.float32)
    router_w = np.asarray(router_w, np.float32)
    routing_bias = np.asarray(routing_bias, np.float32)
    x2 = x.reshape(T, H)

    core_specs = _dispatch(x2, router_w, routing_bias)
    C = max(128, ((max(len(s[1]) for s in core_specs) + 127) // 128) * 128)

    bf = ml_dtypes.bfloat16
    routed_gate = np.asarray(routed_gate, np.float32)
    routed_up = np.asarray(routed_up, np.float32)
    routed_down = np.asarray(routed_down, np.float32)
    gw_s = [_shuffle_gateup(routed_gate[e]) for e in range(E_ROUTED)]
    uw_s = [_shuffle_gateup(routed_up[e]) for e in range(E_ROUTED)]
    dw_b = [np.ascontiguousarray(routed_down[e].astype(bf)) for e in range(E_ROUTED)]
    sgw_s = _shuffle_gateup(np.asarray(shared_gate, np.float32))
    suw_s = _shuffle_gateup(np.asarray(shared_up, np.float32))
    sdw_b = np.ascontiguousarray(np.asarray(shared_down, np.float32).astype(bf))

    in_maps = []
    for c in range(NCORES):
        e, toks = core_specs[c]
        n = len(toks)
        idx_h = np.full((C,), PAD_IDX, np.int32)
        wv_h = np.zeros((C,), np.float32)
        xg = np.zeros((C, H), np.float32)
        if n:
            tok_ids = np.fromiter((t for t, _ in toks), np.int64, n)
            idx_h[:n] = tok_ids
            wv_h[:n] = np.fromiter((wt for _, wt in toks), np.float64, n)
            xg[:n] = x2[tok_ids]
        xt_all = np.concatenate(
            [xg.T, x2[c * SH:(c + 1) * SH].T], axis=1).astype(bf)
        in_maps.append({
            'xt': np.ascontiguousarray(xt_all),
            'idx': idx_h,
            'wv': wv_h,
            'gw': gw_s[e], 'uw': uw_s[e], 'dw': dw_b[e],
            'sgw': sgw_s, 'suw': suw_s, 'sdw': sdw_b,
        })

    nc = _PROG_CACHE.get(C)
    if nc is None:
        nc = _build_program(C)
        _PROG_CACHE[C] = nc

    res = run_bass_kernel_spmd(nc, in_maps, list(range(NCORES)))
    LAST_RESULT = res

    out = np.concatenate([res.results[c]['out'] for c in range(NCORES)], axis=0)
    return out.reshape(B, S, H).astype(np.float32)
